# revision 30
# baseline (speedup 1.0000x reference)
"""Trainium2 Bass kernel for the AgentLoss problem (v2: r-only + fp32r).

Math: for each (l, b) the reference computes the masked cosine-similarity sum
    S = sum_{i != j} <x_i, x_j> / (|x_i| |x_j| + EPS)
over n=1024 agents with c=64 channels, then loss = sum_l mean_b S / (n(n-1)).

EPS (1e-5) is ~1.6e-7 of the denominator (|x_i||x_j| ~ 64), so dropping it
changes the loss by ~1e-7 relative (validated vs fp64).  With r_i = 1/|x_i|
the double sum is then separable:
    S = |sum_i x_i r_i|^2 - sum_i |x_i|^2 r_i^2 = |s|^2 - n
so each (l, b) pair only needs the weighted column-sum s (64 floats).

Device work per (l, b) pair: squares (ACT/GpSimd, bf16 out), per-agent
segmented reduce (DVE, bf16 in -> fp32 out), r^2 = 1/msq (DVE), r = sqrt
(ACT), then two fp32r matmuls contracting the partition axis with 4 r-columns
each (out free size 256 hits the PE's 1-cycle/row fp32r fast path),
accumulating the two 4-chunk halves so the psum diagonal blocks sum to s.
All 8 pairs land in one psum bank [32, 256]; one DVE copy stages it and one
DMA ships it.  The host sums the diagonal blocks and the scalar combine in
float64.

Sharding: data-parallel over batch b - core k takes b in {2k, 2k+1}, i.e.
8 (l, b_local) pairs per core.  Each core returns a [32, 256] block.
"""

from contextlib import ExitStack

import numpy as np

import concourse.bass as bass
from concourse import bacc, mybir
from concourse.bass_utils import run_bass_kernel_spmd

EPS = 1e-5
L, B, N, C = 4, 16, 1024, 64
P = 128            # SBUF partitions
T = N // P         # 8 agent sub-rows per partition
NCORES = 8
BPC = B // NCORES  # b per core
NPAIR = L * BPC    # (l, b_local) pairs per core

F32 = mybir.dt.float32
F32R = mybir.dt.float32r
BF16 = mybir.dt.bfloat16

SPLIT = (6, 7)       # pairs whose input DMA is split in two halves
GP_SQ = (0, 2, 4)    # squares on GpSimd; rest on ACT
ACT_SQ_ORDER = [(1, 0, T), (3, 0, T), (5, 0, T),
                (7, 0, T // 2), (7, T // 2, T)]  # (pair, t0, t1)
GP_SQ_ORDER = [(0, 0, T), (2, 0, T), (4, 0, T),
               (6, 0, T // 2), (6, T // 2, T)]


def build_nc() -> bass.Bass:
    nc = bacc.Bacc("TRN2", target_bir_lowering=False, debug=False, num_devices=NCORES)
    x = nc.declare_dram_parameter("x", [NPAIR, N, C], F32, isOutput=False)
    out = nc.declare_dram_parameter("out", [4, NPAIR * 256], F32, isOutput=True)

    ctx = ExitStack()
    with ctx:
        def sb(name, shape, dt=F32):
            return ctx.enter_context(nc.sbuf_tensor(name, shape, dt))

        xp = [sb(f"xp{j}", [P, T, C]) for j in range(NPAIR)]
        xsq = [sb(f"xsq{j}", [P, T, C], BF16) for j in range(NPAIR)]
        msq = sb("msq", [P, NPAIR * T])
        rw = sb("rw", [P, NPAIR * T])
        stage = sb("stage", [4, NPAIR * 256])
        ps = ctx.enter_context(nc.psum_tensor("ps", [4, NPAIR * 256], F32))

        # input-DMA completion sems: one per dma (split pairs have two)
        dma_keys = []
        for j in range(NPAIR):
            if j in SPLIT:
                dma_keys += [(j, 0), (j, 1)]
            else:
                dma_keys += [(j, None)]
        s_in = {k: nc.alloc_semaphore(f"s_in_{k[0]}_{k[1]}") for k in dma_keys}
        s_sqa = nc.alloc_semaphore("s_sqa")  # ACT squares done (ordered)
        s_sqg = nc.alloc_semaphore("s_sqg")  # GpSimd squares done (ordered)
        s_red = nc.alloc_semaphore("s_red")  # DVE reduces done (ordered)
        s_w = nc.alloc_semaphore("s_w")      # ACT sqrts done (per pair)
        s_mm = nc.alloc_semaphore("s_mm")    # matmul pairs done
        s_st = nc.alloc_semaphore("s_st")    # stage copy done
        s_out = nc.alloc_semaphore("s_out")  # out-DMA receipt (never waited)

        def xv(j, t0, t1):
            return x[j].rearrange("(p t) c -> p t c", p=P)[:, t0:t1, :]

        # ---- Sync: all input DMA issues (in pair order), then the out DMA.
        for j, h in dma_keys:
            t0, t1 = (0, T) if h is None else (h * T // 2, (h + 1) * T // 2)
            eng = nc.sync if j % 2 == 0 else nc.scalar
            eng.dma_start(
                out=xp[j][:, t0:t1, :].bitcast(F32R),
                in_=xv(j, t0, t1).bitcast(F32R),
            ).then_inc(s_in[(j, h)], 16)
        nc.sync.dma_start(
            out=out[:, 0 : 6 * 256], in_=stage[:, 0 : 6 * 256]
        )._wait_ge(s_st, 1).then_inc(s_out, 16)
        nc.sync.dma_start(
            out=out[:, 6 * 256 :], in_=stage[:, 6 * 256 :]
        )._wait_ge(s_st, 2).then_inc(s_out, 16)

        # ---- Scalar (ACT): its share of the input DMA issues is emitted
        # first (dma_keys loop above), then squares + per-pair sqrts
        # interleaved in expected-ready order.  The Square/Sqrt act-table
        # load is auto-inserted before the first square.

        act_ops = []  # (kind, args...) in emission order
        # interleave: sq1, sqrt0, sq3, sqrt1, sqrt2, sq5, sqrt3, sqrt4,
        #             sq7h1, sqrt5, sq7h2, sqrt6, sqrt7
        sq_iter = iter(enumerate(ACT_SQ_ORDER, start=1))
        order = ["sq", "sqrt0", "sq", "sqrt1", "sqrt2", "sq", "sqrt3", "sqrt4",
                 "sq", "sqrt5", "sq", "sqrt6", "sqrt7"]
        for tok in order:
            if tok == "sq":
                cnt, (j, t0, t1) = next(sq_iter)
                key = (j, None) if j not in SPLIT else (j, 0 if t0 == 0 else 1)
                nc.scalar.square(
                    xsq[j][:, t0:t1, :], xp[j][:, t0:t1, :]
                )._wait_ge(s_in[key], 16).then_inc(s_sqa)
            else:
                jj = int(tok[4:])
                # r = 1/sqrt(msq) in one ACT op (no DVE reciprocal hop);
                # threshold = DVE reduce count through pair jj
                thr = {0: 1, 1: 2, 2: 3, 3: 4, 4: 5, 5: 6, 6: 8, 7: 10}[jj]
                nc.scalar.activation(
                    rw[:, T * jj : T * (jj + 1)].bitcast(F32R),
                    msq[:, T * jj : T * (jj + 1)],
                    mybir.ActivationFunctionType.Abs_reciprocal_sqrt,
                )._wait_ge(s_red, thr).then_inc(s_w)

        # ---- GpSimd: squares for even pairs (+ halves of pair 6).
        for cnt, (j, t0, t1) in enumerate(GP_SQ_ORDER, start=1):
            key = (j, None) if j not in SPLIT else (j, 0 if t0 == 0 else 1)
            nc.gpsimd.tensor_mul(
                xsq[j][:, t0:t1, :], xp[j][:, t0:t1, :], xp[j][:, t0:t1, :]
            )._wait_ge(s_in[key], 16).then_inc(s_sqg)

        # ---- Vector (DVE): reduces (bf16 in, fp32 out), recips, final copy.
        # reduce completion order: 0,1,2,3,4,5,6h1,6h2,7h1,7h2
        sqa_cnt = {}  # pair-part -> counter value on its square's sem
        for c, (j, t0, t1) in enumerate(ACT_SQ_ORDER, start=1):
            sqa_cnt[(j, t0)] = ("a", c)
        for c, (j, t0, t1) in enumerate(GP_SQ_ORDER, start=1):
            sqa_cnt[(j, t0)] = ("g", c)

        red_n = 0

        def reduce_part(j, t0, t1):
            nonlocal red_n
            eng, c = sqa_cnt[(j, t0)]
            sem = s_sqa if eng == "a" else s_sqg
            nc.vector.tensor_reduce(
                out=msq[:, T * j + t0 : T * j + t1],
                in_=xsq[j][:, t0:t1, :],
                axis=mybir.AxisListType.X,
                op=mybir.AluOpType.add,
            )._wait_ge(sem, c).then_inc(s_red)
            red_n += 1

        for j in range(NPAIR):
            if j in SPLIT:
                reduce_part(j, 0, T // 2)
                reduce_part(j, T // 2, T)
            else:
                reduce_part(j, 0, T)

        nc.vector.tensor_copy(stage[:, 0 : 6 * 256], ps[:, 0 : 6 * 256])._wait_ge(
            s_mm, 6
        ).then_inc(s_st)
        nc.vector.tensor_copy(
            stage[:, 6 * 256 :], ps[:, 6 * 256 :]
        )._wait_ge(s_mm, NPAIR).then_inc(s_st)

        # ---- Tensor (PE): two fp32r matmuls per pair, accumulating the two
        # 4-chunk halves; psum rows 0..3, diagonal 64-blocks sum to s.
        for j in range(NPAIR):
            for h in range(2):
                mm = nc.tensor.matmul(
                    ps[:, 256 * j : 256 * (j + 1)],
                    rw[:, T * j + 4 * h : T * j + 4 * h + 4].bitcast(F32R),
                    xp[j][:, 4 * h : 4 * h + 4, :].bitcast(F32R),
                    start=(h == 0),
                    stop=(h == 1),
                )
                if h == 0:
                    mm._wait_ge(s_w, j + 1)
                else:
                    mm.then_inc(s_mm)

    nc.compile()
    return nc


_NC_CACHE = None


def _get_nc():
    global _NC_CACHE
    if _NC_CACHE is None:
        _NC_CACHE = build_nc()
    return _NC_CACHE


def run_cores(x_full: np.ndarray, trace: bool = False):
    """Shard, run on 8 NeuronCores, return (per-core out blocks, results obj)."""
    nc = _get_nc()
    in_maps = []
    for k in range(NCORES):
        shard = np.ascontiguousarray(
            x_full[:, BPC * k : BPC * (k + 1)].reshape(NPAIR, N, C)
        )
        in_maps.append({"x": shard})
    res = run_bass_kernel_spmd(nc, in_maps, list(range(NCORES)), trace=trace)
    outs = [res.results[k]["out"] for k in range(NCORES)]
    return outs, res


def reduce_host(outs) -> np.ndarray:
    total = 0.0
    for blk in outs:
        blk = blk.astype(np.float64)
        for j in range(NPAIR):
            rows = blk[:, 256 * j : 256 * (j + 1)]  # [4, 256]
            s = sum(rows[f, 64 * f : 64 * (f + 1)] for f in range(4))
            total += np.dot(s, s) - float(N)
    loss = total / (N * (N - 1)) / B
    return np.array(loss, dtype=np.float32)


def kernel(updated_agents: np.ndarray) -> np.ndarray:
    outs, _ = run_cores(np.asarray(updated_agents))
    return reduce_host(outs)


# revision 33
# speedup vs baseline: 1.0545x; 1.0545x over previous
"""Trainium2 Bass kernel for the AgentLoss problem (v2: r-only + fp32r).

Math: for each (l, b) the reference computes the masked cosine-similarity sum
    S = sum_{i != j} <x_i, x_j> / (|x_i| |x_j| + EPS)
over n=1024 agents with c=64 channels, then loss = sum_l mean_b S / (n(n-1)).

EPS (1e-5) is ~1.6e-7 of the denominator (|x_i||x_j| ~ 64), so dropping it
changes the loss by ~1e-7 relative (validated vs fp64).  With r_i = 1/|x_i|
the double sum is then separable:
    S = |sum_i x_i r_i|^2 - sum_i |x_i|^2 r_i^2 = |s|^2 - n
so each (l, b) pair only needs the weighted column-sum s (64 floats).

Device work per (l, b) pair: squares (ACT/GpSimd, bf16 out), per-agent
segmented reduce (DVE, bf16 in -> fp32 out), r^2 = 1/msq (DVE), r = sqrt
(ACT), then two fp32r matmuls contracting the partition axis with 4 r-columns
each (out free size 256 hits the PE's 1-cycle/row fp32r fast path),
accumulating the two 4-chunk halves so the psum diagonal blocks sum to s.
All 8 pairs land in one psum bank [32, 256]; one DVE copy stages it and one
DMA ships it.  The host sums the diagonal blocks and the scalar combine in
float64.

Sharding: data-parallel over batch b - core k takes b in {2k, 2k+1}, i.e.
8 (l, b_local) pairs per core.  Each core returns a [32, 256] block.
"""

from contextlib import ExitStack

import numpy as np

import concourse.bass as bass
from concourse import bacc, mybir
from concourse.bass_utils import run_bass_kernel_spmd

EPS = 1e-5
L, B, N, C = 4, 16, 1024, 64
P = 128            # SBUF partitions
T = N // P         # 8 agent sub-rows per partition
NCORES = 8
BPC = B // NCORES  # b per core
NPAIR = L * BPC    # (l, b_local) pairs per core

F32 = mybir.dt.float32
F32R = mybir.dt.float32r
BF16 = mybir.dt.bfloat16

SPLIT = (6, 7)       # pairs whose input DMA is split in two halves
GP_SQ = (0, 2, 4)    # squares on GpSimd; rest on ACT
ACT_SQ_ORDER = [(1, 0, T), (3, 0, T), (5, 0, T),
                (7, 0, T // 2), (7, T // 2, T)]  # (pair, t0, t1)
GP_SQ_ORDER = [(0, 0, T), (2, 0, T), (4, 0, T),
               (6, 0, T // 2), (6, T // 2, T)]


def build_nc() -> bass.Bass:
    nc = bacc.Bacc("TRN2", target_bir_lowering=False, debug=False, num_devices=NCORES)
    x = nc.declare_dram_parameter("x", [NPAIR, N, C], F32, isOutput=False)
    out = nc.declare_dram_parameter("out", [4, NPAIR * 256], F32, isOutput=True)

    ctx = ExitStack()
    with ctx:
        def sb(name, shape, dt=F32):
            return ctx.enter_context(nc.sbuf_tensor(name, shape, dt))

        xp = [sb(f"xp{j}", [P, T, C]) for j in range(NPAIR)]
        xsq = [sb(f"xsq{j}", [P, T, C], BF16) for j in range(NPAIR)]
        msq = sb("msq", [P, NPAIR * T])
        rsq = sb("rsq", [P, NPAIR * T])
        rw = sb("rw", [P, NPAIR * T])
        stage = sb("stage", [4, NPAIR * 256])
        ps = ctx.enter_context(nc.psum_tensor("ps", [4, NPAIR * 256], F32))

        # input-DMA completion sems: one per dma (split pairs have two)
        dma_keys = []
        for j in range(NPAIR):
            if j in SPLIT:
                dma_keys += [(j, 0), (j, 1)]
            else:
                dma_keys += [(j, None)]
        s_in = {k: nc.alloc_semaphore(f"s_in_{k[0]}_{k[1]}") for k in dma_keys}
        s_sqa = nc.alloc_semaphore("s_sqa")  # ACT squares done (ordered)
        s_sqg = nc.alloc_semaphore("s_sqg")  # GpSimd squares done (ordered)
        s_red = nc.alloc_semaphore("s_red")  # DVE reduces done (ordered)
        s_rsq = nc.alloc_semaphore("s_rsq")  # DVE recips done (per pair)
        s_w = nc.alloc_semaphore("s_w")      # ACT sqrts done (per pair)
        s_mm = nc.alloc_semaphore("s_mm")    # matmul pairs done
        s_st = nc.alloc_semaphore("s_st")    # stage copy A done (DVE)
        s_stb = nc.alloc_semaphore("s_stb")  # stage copy B done (ACT)
        s_out = nc.alloc_semaphore("s_out")  # out-DMA receipt (never waited)

        def xv(j, t0, t1):
            return x[j].rearrange("(p t) c -> p t c", p=P)[:, t0:t1, :]

        # ---- Sync: all input DMA issues (in pair order), then the out DMA.
        for j, h in dma_keys:
            t0, t1 = (0, T) if h is None else (h * T // 2, (h + 1) * T // 2)
            eng = nc.sync if j % 2 == 0 else nc.scalar
            eng.dma_start(
                out=xp[j][:, t0:t1, :].bitcast(F32R),
                in_=xv(j, t0, t1).bitcast(F32R),
            ).then_inc(s_in[(j, h)], 16)
        nc.sync.dma_start(
            out=out[:, 0 : 6 * 256], in_=stage[:, 0 : 6 * 256]
        )._wait_ge(s_st, 1).then_inc(s_out, 16)

        # ---- Scalar (ACT): its share of the input DMA issues is emitted
        # first (dma_keys loop above), then squares + per-pair sqrts
        # interleaved in expected-ready order.  The Square/Sqrt act-table
        # load is auto-inserted before the first square.

        act_ops = []  # (kind, args...) in emission order
        # interleave: sq1, sqrt0, sq3, sqrt1, sqrt2, sq5, sqrt3, sqrt4,
        #             sq7h1, sqrt5, sq7h2, sqrt6, sqrt7
        sq_iter = iter(enumerate(ACT_SQ_ORDER, start=1))
        order = ["sq", "sqrt0", "sq", "sqrt1", "sqrt2", "sq", "sqrt3", "sqrt4",
                 "sq", "sqrt5", "sq", "sqrt6", "sqrt7"]
        for tok in order:
            if tok == "sq":
                cnt, (j, t0, t1) = next(sq_iter)
                key = (j, None) if j not in SPLIT else (j, 0 if t0 == 0 else 1)
                nc.scalar.square(
                    xsq[j][:, t0:t1, :], xp[j][:, t0:t1, :]
                )._wait_ge(s_in[key], 16).then_inc(s_sqa)
            else:
                jj = int(tok[4:])
                nc.scalar.activation(
                    rw[:, T * jj : T * (jj + 1)].bitcast(F32R),
                    rsq[:, T * jj : T * (jj + 1)],
                    mybir.ActivationFunctionType.Sqrt,
                )._wait_ge(s_rsq, jj + 1).then_inc(s_w)

        # ---- GpSimd: squares for even pairs (+ halves of pair 6).
        for cnt, (j, t0, t1) in enumerate(GP_SQ_ORDER, start=1):
            key = (j, None) if j not in SPLIT else (j, 0 if t0 == 0 else 1)
            nc.gpsimd.tensor_mul(
                xsq[j][:, t0:t1, :], xp[j][:, t0:t1, :], xp[j][:, t0:t1, :]
            )._wait_ge(s_in[key], 16).then_inc(s_sqg)

        # ---- Vector (DVE): reduces (bf16 in, fp32 out), recips, final copy.
        # reduce completion order: 0,1,2,3,4,5,6h1,6h2,7h1,7h2
        sqa_cnt = {}  # pair-part -> counter value on its square's sem
        for c, (j, t0, t1) in enumerate(ACT_SQ_ORDER, start=1):
            sqa_cnt[(j, t0)] = ("a", c)
        for c, (j, t0, t1) in enumerate(GP_SQ_ORDER, start=1):
            sqa_cnt[(j, t0)] = ("g", c)

        red_n = 0

        def reduce_part(j, t0, t1):
            nonlocal red_n
            eng, c = sqa_cnt[(j, t0)]
            sem = s_sqa if eng == "a" else s_sqg
            nc.vector.tensor_reduce(
                out=msq[:, T * j + t0 : T * j + t1],
                in_=xsq[j][:, t0:t1, :],
                axis=mybir.AxisListType.X,
                op=mybir.AluOpType.add,
            )._wait_ge(sem, c).then_inc(s_red)
            red_n += 1

        def recip(j):
            nc.vector.reciprocal(
                out=rsq[:, T * j : T * (j + 1)], in_=msq[:, T * j : T * (j + 1)]
            )._wait_ge(s_red, red_n).then_inc(s_rsq)

        for j in range(NPAIR):
            if j in SPLIT:
                reduce_part(j, 0, T // 2)
                reduce_part(j, T // 2, T)
            else:
                reduce_part(j, 0, T)
            recip(j)

        nc.vector.tensor_copy(stage[:, 0 : 6 * 256], ps[:, 0 : 6 * 256])._wait_ge(
            s_mm, 6
        ).then_inc(s_st)
        # pairs 6-7 staged on ACT, in parallel with DVE's big copy
        nc.scalar.activation(
            stage[:, 6 * 256 :], ps[:, 6 * 256 :],
            mybir.ActivationFunctionType.Copy,
        )._wait_ge(s_mm, NPAIR).then_inc(s_stb)
        nc.scalar.dma_start(
            out=out[:, 6 * 256 :], in_=stage[:, 6 * 256 :]
        )._wait_ge(s_stb, 1).then_inc(s_out, 16)

        # ---- Tensor (PE): two fp32r matmuls per pair, accumulating the two
        # 4-chunk halves; psum rows 0..3, diagonal 64-blocks sum to s.
        for j in range(NPAIR):
            for h in range(2):
                mm = nc.tensor.matmul(
                    ps[:, 256 * j : 256 * (j + 1)],
                    rw[:, T * j + 4 * h : T * j + 4 * h + 4].bitcast(F32R),
                    xp[j][:, 4 * h : 4 * h + 4, :].bitcast(F32R),
                    start=(h == 0),
                    stop=(h == 1),
                )
                if h == 0:
                    mm._wait_ge(s_w, j + 1)
                else:
                    mm.then_inc(s_mm)

    nc.compile()
    return nc


_NC_CACHE = None


def _get_nc():
    global _NC_CACHE
    if _NC_CACHE is None:
        _NC_CACHE = build_nc()
    return _NC_CACHE


def run_cores(x_full: np.ndarray, trace: bool = False):
    """Shard, run on 8 NeuronCores, return (per-core out blocks, results obj)."""
    nc = _get_nc()
    in_maps = []
    for k in range(NCORES):
        shard = np.ascontiguousarray(
            x_full[:, BPC * k : BPC * (k + 1)].reshape(NPAIR, N, C)
        )
        in_maps.append({"x": shard})
    res = run_bass_kernel_spmd(nc, in_maps, list(range(NCORES)), trace=trace)
    outs = [res.results[k]["out"] for k in range(NCORES)]
    return outs, res


def reduce_host(outs) -> np.ndarray:
    total = 0.0
    for blk in outs:
        blk = blk.astype(np.float64)
        for j in range(NPAIR):
            rows = blk[:, 256 * j : 256 * (j + 1)]  # [4, 256]
            s = sum(rows[f, 64 * f : 64 * (f + 1)] for f in range(4))
            total += np.dot(s, s) - float(N)
    loss = total / (N * (N - 1)) / B
    return np.array(loss, dtype=np.float32)


def kernel(updated_agents: np.ndarray) -> np.ndarray:
    outs, _ = run_cores(np.asarray(updated_agents))
    return reduce_host(outs)
